# revision 21
# baseline (speedup 1.0000x reference)
"""Trainium2 Bass kernel: 3-layer LSTMCell stack + skip dense + log_softmax.

Data-parallel across 8 NeuronCores: batch (8192) sharded into 1024 rows per
core, weights replicated. All matmuls run in a feature-major ("transposed")
layout so the contraction dim sits on SBUF partitions with no on-chip
transposes: activations live as (feature, batch) tiles, weights are
pre-arranged on the host into lhsT slabs. bf16 matmul inputs, fp32
accumulation and elementwise math.
"""

import sys

for _p in ("/opt/trn_rl_repo",):
    if _p not in sys.path:
        sys.path.insert(0, _p)

import ml_dtypes
import numpy as np

N_CORES = 8
B, S, L, M = 8192, 512, 1024, 512
BF16 = ml_dtypes.bfloat16

_CACHE = {}


def _build(bc, reps=1, diag=None):
    """Build + compile the per-core SPMD Bass program (batch slice = bc).

    reps > 1 wraps the whole body in an on-device loop (for timing: device
    wall-time scales with reps while the fixed dispatch overhead does not).
    diag="mm_only" emits just the DMAs+matmuls (wrong outputs; PE-span probe).
    """
    import contextlib

    import concourse.tile as tile
    import concourse.mybir as mybir
    from concourse import bacc

    mm_only = diag == "mm_only"

    f32 = mybir.dt.float32
    bf16 = mybir.dt.bfloat16
    AF = mybir.ActivationFunctionType

    nch = max(1, bc // 512)  # batch chunks (PSUM free dim <= 512 fp32)
    ch = bc // nch
    nbt = bc // 128          # batch tiles for the final batch-major stage

    nc = bacc.Bacc(None, target_bir_lowering=False, debug=False,
                   num_devices=N_CORES)

    KT = [12, 16, 16]  # contraction k-tiles per layer ((S+L)/128, (L+L)/128 x2)

    xT = nc.dram_tensor("xT", [4, 128, bc], bf16, kind="ExternalInput")
    hT, cT, W, bias, yT, ncT = [], [], [], [], [], []
    for li in range(3):
        hT.append(nc.dram_tensor(f"h{li + 1}T", [8, 128, bc], bf16, kind="ExternalInput"))
        cT.append(nc.dram_tensor(f"c{li + 1}T", [8, 128, bc], f32, kind="ExternalInput"))
        W.append(nc.dram_tensor(f"W{li + 1}", [8, 128, 4, KT[li], 128], bf16, kind="ExternalInput"))
        bias.append(nc.dram_tensor(f"b{li + 1}", [128, 32], f32, kind="ExternalInput"))
        yT.append(nc.dram_tensor(f"y{li + 1}T", [8, 128, bc], f32, kind="ExternalOutput"))
        ncT.append(nc.dram_tensor(f"nc{li + 1}T", [8, 128, bc], f32, kind="ExternalOutput"))
    Wd = nc.dram_tensor("Wd", [8, 128, 4, 128], bf16, kind="ExternalInput")
    bd = nc.dram_tensor("bd", [128, 8], f32, kind="ExternalInput")
    WmT = nc.dram_tensor("WmT", [128, 8, M], bf16, kind="ExternalInput")
    bm = nc.dram_tensor("bm", [1, M], bf16, kind="ExternalInput")
    MP = nc.dram_tensor("mp", [nbt, 128, M], f32, kind="ExternalOutput")

    with tile.TileContext(nc) as tc:
        with tc.tile_pool(name="sb", bufs=1) as sb, \
             tc.tile_pool(name="ps", bufs=2, space="PSUM") as pspool, \
             (tc.For_i(0, reps, 1) if reps > 1 else contextlib.nullcontext()):

            # ---- persistent small/constant tiles -------------------------
            xt = []
            for kt in range(4):
                t = sb.tile([128, bc], bf16, tag=f"x{kt}", name=f"xt{kt}")
                nc.sync.dma_start(out=t, in_=xT[kt])
                xt.append(t)
            b_t = []
            for li in range(3):
                t = sb.tile([128, 32], f32, tag=f"bias{li}", name=f"bt{li}")
                nc.sync.dma_start(out=t, in_=bias[li][:, :])
                b_t.append(t)
            bd_t = sb.tile([128, 8], f32, tag="bd", name="bd_t")
            nc.sync.dma_start(out=bd_t, in_=bd[:, :])
            wm_t = sb.tile([128, 8, M], bf16, tag="wm", name="wm_t")
            nc.sync.dma_start(out=wm_t, in_=WmT[:, :, :])
            bm_t = sb.tile([1, M], bf16, tag="bm", name="bm_t")
            nc.sync.dma_start(out=bm_t, in_=bm[:, :])
            ones_t = sb.tile([1, 128], bf16, tag="ones", name="ones_t")
            nc.vector.memset(ones_t, 1.0)

            # ---- the three LSTM cells ------------------------------------
            ybf_prev = None
            mids = []
            for li in range(3):
                ht = []
                for kt in range(8):
                    t = sb.tile([128, bc], bf16, tag=f"h{kt}", bufs=2,
                                name=f"h{li}_{kt}")
                    nc.sync.dma_start(out=t, in_=hT[li][kt])
                    ht.append(t)
                # h-tiles first: the previous layer's y tiles become ready
                # lb-by-lb, so consume them late in each accumulation group
                rhs = ht + (xt if li == 0 else ybf_prev)
                K = KT[li]
                ybf_new = []
                for lb in range(8):
                    w = sb.tile([128, 4, K, 128], bf16, tag="wslab", bufs=2,
                                name=f"w{li}_{lb}")
                    nc.sync.dma_start(out=w, in_=W[li][lb])
                    if not mm_only:
                        c_t = sb.tile([128, bc], f32, tag="c", bufs=3,
                                      name=f"c{li}_{lb}")
                        nc.sync.dma_start(out=c_t, in_=cT[li][lb])
                    cn = sb.tile([128, bc], f32, tag="cnew", bufs=2,
                                 name=f"cn{li}_{lb}")
                    hn = sb.tile([128, bc], f32, tag="hnew", bufs=2,
                                 name=f"hn{li}_{lb}")
                    yb = sb.tile([128, bc], bf16, tag=f"ybf{lb}", bufs=2,
                                 name=f"yb{li}_{lb}")
                    if mm_only:
                        nc.vector.memset(yb, 0.01)
                    for n in range(nch):
                        sl = slice(n * ch, (n + 1) * ch)
                        ps = [pspool.tile([128, ch], f32, tag=f"ps{g}",
                                          bufs=2, name=f"ps{li}_{lb}_{n}_{g}")
                              for g in range(4)]
                        for g in range(4):
                            for kt in range(K):
                                nc.tensor.matmul(ps[g], w[:, g, kt, :],
                                                 rhs[kt][:, sl],
                                                 start=(kt == 0),
                                                 stop=(kt == K - 1))
                        if mm_only:
                            continue
                        bb = b_t[li]
                        i_s = sb.tile([128, ch], f32, tag="i_s", bufs=2, name=f"i{li}_{lb}_{n}")
                        nc.scalar.activation(out=i_s, in_=ps[0], func=AF.Sigmoid,
                                             bias=bb[:, lb * 4 + 0:lb * 4 + 1])
                        f_s = sb.tile([128, ch], f32, tag="f_s", bufs=2, name=f"f{li}_{lb}_{n}")
                        nc.scalar.activation(out=f_s, in_=ps[1], func=AF.Sigmoid,
                                             bias=bb[:, lb * 4 + 1:lb * 4 + 2])
                        g_t = sb.tile([128, ch], f32, tag="g_t", bufs=2, name=f"g{li}_{lb}_{n}")
                        nc.scalar.activation(out=g_t, in_=ps[2], func=AF.Tanh,
                                             bias=bb[:, lb * 4 + 2:lb * 4 + 3])
                        o_s = sb.tile([128, ch], f32, tag="o_s", bufs=2, name=f"o{li}_{lb}_{n}")
                        nc.scalar.activation(out=o_s, in_=ps[3], func=AF.Sigmoid,
                                             bias=bb[:, lb * 4 + 3:lb * 4 + 4])
                        fc = sb.tile([128, ch], f32, tag="fc", bufs=2, name=f"fc{li}_{lb}_{n}")
                        nc.vector.tensor_mul(fc, f_s, c_t[:, sl])
                        ig = sb.tile([128, ch], f32, tag="ig", bufs=2, name=f"ig{li}_{lb}_{n}")
                        nc.vector.tensor_mul(ig, i_s, g_t)
                        nc.vector.tensor_add(cn[:, sl], fc, ig)
                        tct = sb.tile([128, ch], f32, tag="tct", bufs=2, name=f"tc{li}_{lb}_{n}")
                        nc.scalar.activation(out=tct, in_=cn[:, sl], func=AF.Tanh)
                        nc.vector.tensor_mul(hn[:, sl], o_s, tct)
                    if not mm_only:
                        nc.vector.tensor_copy(out=yb, in_=hn)  # fp32 -> bf16
                        nc.sync.dma_start(out=ncT[li][lb], in_=cn)
                        nc.sync.dma_start(out=yT[li][lb], in_=hn)
                    ybf_new.append(yb)

                    if li == 2:
                        # interleave mid = x @ Wd.T + bd + y3 into layer 3's
                        # loop so its matmuls fill PE time while the cell's
                        # elementwise tail runs
                        wd_t = sb.tile([128, 4, 128], bf16, tag="wd", bufs=2,
                                       name=f"wd{lb}")
                        nc.sync.dma_start(out=wd_t, in_=Wd[lb])
                        mid = sb.tile([128, bc], bf16, tag=f"ybf{lb}", bufs=2,
                                      name=f"mid{lb}")
                        if mm_only:
                            nc.vector.memset(mid, 0.01)
                        for n in range(nch):
                            sl = slice(n * ch, (n + 1) * ch)
                            psd = pspool.tile([128, ch], f32, tag="ps0",
                                              bufs=2, name=f"psd{lb}_{n}")
                            for kt in range(4):
                                nc.tensor.matmul(psd, wd_t[:, kt, :],
                                                 xt[kt][:, sl],
                                                 start=(kt == 0), stop=(kt == 3))
                            if mm_only:
                                continue
                            m1 = sb.tile([128, ch], f32, tag="m1", bufs=2,
                                         name=f"m1_{lb}_{n}")
                            nc.vector.tensor_add(m1, psd, yb[:, sl])
                            nc.vector.tensor_scalar_add(mid[:, sl], m1,
                                                        bd_t[:, lb:lb + 1])
                        mids.append(mid)
                ybf_prev = ybf_new

            # ---- motors (batch-major) + log_softmax ----------------------
            for bt in range(nbt):
                psm = pspool.tile([128, M], f32, tag="ps1", bufs=2, name=f"psm{bt}")
                for kt in range(8):
                    nc.tensor.matmul(psm, mids[kt][:, bt * 128:(bt + 1) * 128],
                                     wm_t[:, kt, :], start=(kt == 0),
                                     stop=False)
                nc.tensor.matmul(psm, ones_t, bm_t, start=False, stop=True)
                if mm_only:
                    continue
                mx = sb.tile([128, 1], f32, tag="mx", bufs=2, name=f"mx{bt}")
                nc.vector.tensor_reduce(mx, psm, axis=mybir.AxisListType.X,
                                        op=mybir.AluOpType.max, negate=True)
                e_t = sb.tile([128, M], f32, tag="e_t", bufs=2, name=f"e{bt}")
                se = sb.tile([128, 1], f32, tag="se", bufs=2, name=f"se{bt}")
                nc.scalar.activation(out=e_t, in_=psm, func=AF.Exp, bias=mx,
                                     accum_out=se)
                ls = sb.tile([128, 1], f32, tag="ls", bufs=2, name=f"ls{bt}")
                nc.scalar.activation(out=ls, in_=se, func=AF.Ln)
                sh = sb.tile([128, 1], f32, tag="sh", bufs=2, name=f"sh{bt}")
                nc.vector.tensor_sub(sh, mx, ls)
                mp_t = sb.tile([128, M], f32, tag="mp_t", bufs=2,
                               name=f"mp{bt}")
                nc.vector.tensor_scalar_add(mp_t, psm, sh)
                nc.sync.dma_start(out=MP[bt], in_=mp_t)

    nc.compile()
    return nc


def get_nc(bc=B // N_CORES, reps=1, diag=None):
    key = ("nc", bc, reps, diag)
    if key not in _CACHE:
        _CACHE[key] = _build(bc, reps, diag)
    return _CACHE[key]


def _arrange_gate_weights(Wih, Whh, kk):
    """-> (8, 128, 4, kk, 128) bf16 lhsT slabs: [lb, p, gate, kt, j] =
    Wcat[gate*1024 + lb*128 + j, kt*128 + p].

    Whh first: the kernel consumes the h-side k-tiles before the x/y-side
    ones (see `rhs = ht + ...` in _build)."""
    Wcat = np.concatenate([Whh, Wih], axis=1)  # (4096, K)
    a = Wcat.reshape(4, 8, 128, kk, 128)       # [gi, lb, j, kt, p]
    return np.ascontiguousarray(a.transpose(1, 4, 0, 3, 2)).astype(BF16)


def prep_inputs(inputs, bc=B // N_CORES):
    """Host-side prep: transpose/rearrange/cast. Returns per-core in_maps."""
    f32 = np.float32
    ncore = B // bc

    def featmaj(a, dt):  # (B, F) -> (F/128, 128, B) in dtype dt
        a = np.ascontiguousarray(np.asarray(a, f32).T)
        if dt is not f32:
            a = a.astype(dt)
        return a.reshape(a.shape[0] // 128, 128, B)

    common = {}
    common["W1"] = _arrange_gate_weights(inputs["Wih1"], inputs["Whh1"], 12)
    common["W2"] = _arrange_gate_weights(inputs["Wih2"], inputs["Whh2"], 16)
    common["W3"] = _arrange_gate_weights(inputs["Wih3"], inputs["Whh3"], 16)
    for li in ("1", "2", "3"):
        bsum = (np.asarray(inputs[f"bih{li}"], f32)
                + np.asarray(inputs[f"bhh{li}"], f32))
        common[f"b{li}"] = np.ascontiguousarray(
            bsum.reshape(4, 8, 128).transpose(2, 1, 0).reshape(128, 32))
    wd = np.asarray(inputs["Wd"], f32).reshape(8, 128, 4, 128)
    common["Wd"] = np.ascontiguousarray(wd.transpose(0, 3, 2, 1)).astype(BF16)
    common["bd"] = np.ascontiguousarray(
        np.asarray(inputs["bd"], f32).reshape(8, 128).T)
    wm = np.ascontiguousarray(np.asarray(inputs["Wm"], f32).T)  # (1024, 512)
    common["WmT"] = np.ascontiguousarray(
        wm.reshape(8, 128, M).transpose(1, 0, 2)).astype(BF16)
    common["bm"] = np.asarray(inputs["bm"], f32).reshape(1, M).astype(BF16)

    sharded = {"xT": featmaj(inputs["x"], BF16)}
    for li in ("1", "2", "3"):
        sharded[f"h{li}T"] = featmaj(inputs[f"h{li}"], BF16)
        sharded[f"c{li}T"] = featmaj(inputs[f"c{li}"], f32)

    in_maps = []
    for c in range(ncore):
        m = dict(common)
        for k, v in sharded.items():
            m[k] = np.ascontiguousarray(v[:, :, c * bc:(c + 1) * bc])
        in_maps.append(m)
    return in_maps


def assemble_outputs(results, bc=B // N_CORES):
    ncore = len(results)
    f32 = np.float32

    def gather(name):
        out = np.empty((ncore * bc, L), f32)
        for c in range(ncore):
            out[c * bc:(c + 1) * bc] = results[c][name].reshape(L, bc).T
        return out

    mp = np.concatenate([results[c]["mp"].reshape(bc, M) for c in range(ncore)],
                        axis=0)
    return (mp, gather("y1T"), gather("nc1T"), gather("y2T"), gather("nc2T"),
            gather("y3T"), gather("nc3T"))


def run_on_cores(in_maps, bc=B // N_CORES):
    from concourse.bass_utils import run_bass_kernel_spmd
    nc = get_nc(bc)
    res = run_bass_kernel_spmd(nc, in_maps, list(range(len(in_maps))))
    return res.results


def kernel(**inputs):
    bc = B // N_CORES
    in_maps = prep_inputs(inputs, bc)
    results = run_on_cores(in_maps, bc)
    return assemble_outputs(results, bc)


# revision 26
# speedup vs baseline: 1.1110x; 1.1110x over previous
"""Trainium2 Bass kernel: 3-layer LSTMCell stack + skip dense + log_softmax.

Data-parallel across 8 NeuronCores: batch (8192) sharded into 1024 rows per
core, weights replicated. All matmuls run in a feature-major ("transposed")
layout so the contraction dim sits on SBUF partitions with no on-chip
transposes: activations live as (feature, batch) tiles, weights are
pre-arranged on the host into lhsT slabs. bf16 matmul inputs, fp32
accumulation and elementwise math.
"""

import sys

for _p in ("/opt/trn_rl_repo",):
    if _p not in sys.path:
        sys.path.insert(0, _p)

import ml_dtypes
import numpy as np

N_CORES = 8
B, S, L, M = 8192, 512, 1024, 512
BF16 = ml_dtypes.bfloat16

_CACHE = {}


def _build(bc, reps=1, diag=None):
    """Build + compile the per-core SPMD Bass program (batch slice = bc).

    reps > 1 wraps the whole body in an on-device loop (for timing: device
    wall-time scales with reps while the fixed dispatch overhead does not).
    diag="mm_only" emits just the DMAs+matmuls (wrong outputs; PE-span probe).
    """
    import contextlib

    import concourse.tile as tile
    import concourse.mybir as mybir
    from concourse import bacc

    mm_only = diag == "mm_only"

    f32 = mybir.dt.float32
    bf16 = mybir.dt.bfloat16
    AF = mybir.ActivationFunctionType

    nch = max(1, bc // 512)  # batch chunks (PSUM free dim <= 512 fp32)
    ch = bc // nch
    nbt = bc // 128          # batch tiles for the final batch-major stage

    nc = bacc.Bacc(None, target_bir_lowering=False, debug=False,
                   num_devices=N_CORES)

    KT = [12, 16, 16]  # contraction k-tiles per layer ((S+L)/128, (L+L)/128 x2)

    xT = nc.dram_tensor("xT", [4, 128, bc], bf16, kind="ExternalInput")
    hT, cT, W, bias, yT, ncT = [], [], [], [], [], []
    for li in range(3):
        hT.append(nc.dram_tensor(f"h{li + 1}T", [8, 128, bc], bf16, kind="ExternalInput"))
        cT.append(nc.dram_tensor(f"c{li + 1}T", [8, 128, bc], f32, kind="ExternalInput"))
        W.append(nc.dram_tensor(f"W{li + 1}", [8, 128, 4, KT[li], 128], bf16, kind="ExternalInput"))
        bias.append(nc.dram_tensor(f"b{li + 1}", [128, 32], f32, kind="ExternalInput"))
        yT.append(nc.dram_tensor(f"y{li + 1}T", [8, 128, bc], f32, kind="ExternalOutput"))
        ncT.append(nc.dram_tensor(f"nc{li + 1}T", [8, 128, bc], f32, kind="ExternalOutput"))
    # host-folded dense stages: motors = x @ (Wm@Wd).T + y3 @ Wm.T + bias2
    WdmT = nc.dram_tensor("WdmT", [128, 4, M], bf16, kind="ExternalInput")
    WmT = nc.dram_tensor("WmT", [128, 8, M], bf16, kind="ExternalInput")
    bm = nc.dram_tensor("bm", [1, M], bf16, kind="ExternalInput")
    MP = nc.dram_tensor("mp", [nbt, 128, M], f32, kind="ExternalOutput")

    with tile.TileContext(nc) as tc:
        with tc.tile_pool(name="sb", bufs=1) as sb, \
             tc.tile_pool(name="ps", bufs=2, space="PSUM") as pspool, \
             (tc.For_i(0, reps, 1) if reps > 1 else contextlib.nullcontext()):

            # ---- persistent small/constant tiles -------------------------
            xt = []
            for kt in range(4):
                t = sb.tile([128, bc], bf16, tag=f"x{kt}", name=f"xt{kt}")
                nc.sync.dma_start(out=t, in_=xT[kt])
                xt.append(t)
            b_t = []
            for li in range(3):
                t = sb.tile([128, 32], f32, tag=f"bias{li}", name=f"bt{li}")
                nc.sync.dma_start(out=t, in_=bias[li][:, :])
                b_t.append(t)
            wdm_t = sb.tile([128, 4, M], bf16, tag="wdm", name="wdm_t")
            nc.sync.dma_start(out=wdm_t, in_=WdmT[:, :, :])
            wm_t = sb.tile([128, 8, M], bf16, tag="wm", name="wm_t")
            nc.sync.dma_start(out=wm_t, in_=WmT[:, :, :])
            bm_t = sb.tile([1, M], bf16, tag="bm", name="bm_t")
            nc.sync.dma_start(out=bm_t, in_=bm[:, :])
            ones_t = sb.tile([1, 128], bf16, tag="ones", name="ones_t")
            nc.vector.memset(ones_t, 1.0)

            # ---- the three LSTM cells ------------------------------------
            ybf_prev = None
            mids = []
            for li in range(3):
                ht = []
                for kt in range(8):
                    t = sb.tile([128, bc], bf16, tag=f"h{kt}", bufs=2,
                                name=f"h{li}_{kt}")
                    nc.sync.dma_start(out=t, in_=hT[li][kt])
                    ht.append(t)
                # h-tiles first: the previous layer's y tiles become ready
                # lb-by-lb, so consume them late in each accumulation group
                rhs = ht + (xt if li == 0 else ybf_prev)
                K = KT[li]
                ybf_new = []
                for lb in range(8):
                    w = sb.tile([128, 4, K, 128], bf16, tag="wslab", bufs=2,
                                name=f"w{li}_{lb}")
                    nc.sync.dma_start(out=w, in_=W[li][lb])
                    if not mm_only:
                        c_t = sb.tile([128, bc], f32, tag="c", bufs=3,
                                      name=f"c{li}_{lb}")
                        nc.sync.dma_start(out=c_t, in_=cT[li][lb])
                    cn = sb.tile([128, bc], f32, tag="cnew", bufs=2,
                                 name=f"cn{li}_{lb}")
                    hn = sb.tile([128, bc], f32, tag="hnew", bufs=2,
                                 name=f"hn{li}_{lb}")
                    yb = sb.tile([128, bc], bf16, tag=f"ybf{lb}", bufs=2,
                                 name=f"yb{li}_{lb}")
                    if mm_only:
                        nc.vector.memset(yb, 0.01)
                    for n in range(nch):
                        sl = slice(n * ch, (n + 1) * ch)
                        ps = [pspool.tile([128, ch], f32, tag=f"ps{g}",
                                          bufs=2, name=f"ps{li}_{lb}_{n}_{g}")
                              for g in range(4)]
                        for g in range(4):
                            for kt in range(K):
                                nc.tensor.matmul(ps[g], w[:, g, kt, :],
                                                 rhs[kt][:, sl],
                                                 start=(kt == 0),
                                                 stop=(kt == K - 1))
                        if mm_only:
                            continue
                        bb = b_t[li]
                        i_s = sb.tile([128, ch], f32, tag="i_s", bufs=2, name=f"i{li}_{lb}_{n}")
                        nc.scalar.activation(out=i_s, in_=ps[0], func=AF.Sigmoid,
                                             bias=bb[:, lb * 4 + 0:lb * 4 + 1])
                        f_s = sb.tile([128, ch], f32, tag="f_s", bufs=2, name=f"f{li}_{lb}_{n}")
                        nc.scalar.activation(out=f_s, in_=ps[1], func=AF.Sigmoid,
                                             bias=bb[:, lb * 4 + 1:lb * 4 + 2])
                        g_t = sb.tile([128, ch], f32, tag="g_t", bufs=2, name=f"g{li}_{lb}_{n}")
                        nc.scalar.activation(out=g_t, in_=ps[2], func=AF.Tanh,
                                             bias=bb[:, lb * 4 + 2:lb * 4 + 3])
                        o_s = sb.tile([128, ch], f32, tag="o_s", bufs=2, name=f"o{li}_{lb}_{n}")
                        nc.scalar.activation(out=o_s, in_=ps[3], func=AF.Sigmoid,
                                             bias=bb[:, lb * 4 + 3:lb * 4 + 4])
                        fc = sb.tile([128, ch], f32, tag="fc", bufs=2, name=f"fc{li}_{lb}_{n}")
                        nc.vector.tensor_mul(fc, f_s, c_t[:, sl])
                        ig = sb.tile([128, ch], f32, tag="ig", bufs=2, name=f"ig{li}_{lb}_{n}")
                        nc.vector.tensor_mul(ig, i_s, g_t)
                        nc.vector.tensor_add(cn[:, sl], fc, ig)
                        tct = sb.tile([128, ch], f32, tag="tct", bufs=2, name=f"tc{li}_{lb}_{n}")
                        nc.scalar.activation(out=tct, in_=cn[:, sl], func=AF.Tanh)
                        nc.vector.tensor_mul(hn[:, sl], o_s, tct)
                    if not mm_only:
                        nc.vector.tensor_copy(out=yb, in_=hn)  # fp32 -> bf16
                        nc.sync.dma_start(out=ncT[li][lb], in_=cn)
                        nc.sync.dma_start(out=yT[li][lb], in_=hn)
                    ybf_new.append(yb)
                ybf_prev = ybf_new
            y3bf = ybf_prev

            # ---- motors (batch-major) + log_softmax ----------------------
            # motors = x @ (Wm@Wd).T + y3 @ Wm.T + (Wm@bd + bm), folded on
            # the host, so the x-side accumulation is ready from the start
            # and y3 tiles are consumed in the order layer 3 produces them.
            for bt in range(nbt):
                bsl = slice(bt * 128, (bt + 1) * 128)
                psm = pspool.tile([128, M], f32, tag="ps1", bufs=2, name=f"psm{bt}")
                for kt in range(4):
                    nc.tensor.matmul(psm, xt[kt][:, bsl], wdm_t[:, kt, :],
                                     start=(kt == 0), stop=False)
                for kt in range(8):
                    nc.tensor.matmul(psm, y3bf[kt][:, bsl], wm_t[:, kt, :],
                                     start=False, stop=False)
                nc.tensor.matmul(psm, ones_t, bm_t, start=False, stop=True)
                if mm_only:
                    continue
                mx = sb.tile([128, 1], f32, tag="mx", bufs=2, name=f"mx{bt}")
                nc.vector.tensor_reduce(mx, psm, axis=mybir.AxisListType.X,
                                        op=mybir.AluOpType.max, negate=True)
                e_t = sb.tile([128, M], f32, tag="e_t", bufs=2, name=f"e{bt}")
                se = sb.tile([128, 1], f32, tag="se", bufs=2, name=f"se{bt}")
                nc.scalar.activation(out=e_t, in_=psm, func=AF.Exp, bias=mx,
                                     accum_out=se)
                ls = sb.tile([128, 1], f32, tag="ls", bufs=2, name=f"ls{bt}")
                nc.scalar.activation(out=ls, in_=se, func=AF.Ln)
                sh = sb.tile([128, 1], f32, tag="sh", bufs=2, name=f"sh{bt}")
                nc.vector.tensor_sub(sh, mx, ls)
                mp_t = sb.tile([128, M], f32, tag="mp_t", bufs=2,
                               name=f"mp{bt}")
                nc.vector.tensor_scalar_add(mp_t, psm, sh)
                nc.sync.dma_start(out=MP[bt], in_=mp_t)

    nc.compile()
    return nc


def get_nc(bc=B // N_CORES, reps=1, diag=None):
    key = ("nc", bc, reps, diag)
    if key not in _CACHE:
        _CACHE[key] = _build(bc, reps, diag)
    return _CACHE[key]


def _arrange_gate_weights(Wih, Whh, kk):
    """-> (8, 128, 4, kk, 128) bf16 lhsT slabs: [lb, p, gate, kt, j] =
    Wcat[gate*1024 + lb*128 + j, kt*128 + p].

    Whh first: the kernel consumes the h-side k-tiles before the x/y-side
    ones (see `rhs = ht + ...` in _build)."""
    Wcat = np.concatenate([Whh, Wih], axis=1)  # (4096, K)
    a = Wcat.reshape(4, 8, 128, kk, 128)       # [gi, lb, j, kt, p]
    return np.ascontiguousarray(a.transpose(1, 4, 0, 3, 2)).astype(BF16)


def prep_inputs(inputs, bc=B // N_CORES):
    """Host-side prep: transpose/rearrange/cast. Returns per-core in_maps."""
    f32 = np.float32
    ncore = B // bc

    def featmaj(a, dt):  # (B, F) -> (F/128, 128, B) in dtype dt
        a = np.ascontiguousarray(np.asarray(a, f32).T)
        if dt is not f32:
            a = a.astype(dt)
        return a.reshape(a.shape[0] // 128, 128, B)

    common = {}
    common["W1"] = _arrange_gate_weights(inputs["Wih1"], inputs["Whh1"], 12)
    common["W2"] = _arrange_gate_weights(inputs["Wih2"], inputs["Whh2"], 16)
    common["W3"] = _arrange_gate_weights(inputs["Wih3"], inputs["Whh3"], 16)
    for li in ("1", "2", "3"):
        bsum = (np.asarray(inputs[f"bih{li}"], f32)
                + np.asarray(inputs[f"bhh{li}"], f32))
        common[f"b{li}"] = np.ascontiguousarray(
            bsum.reshape(4, 8, 128).transpose(2, 1, 0).reshape(128, 32))
    wd = np.asarray(inputs["Wd"], np.float64)   # (1024, 512)
    wm = np.asarray(inputs["Wm"], np.float64)   # (512, 1024)
    wdm = (wm @ wd).T.astype(f32)               # (512, 512) = (Wm@Wd).T
    common["WdmT"] = np.ascontiguousarray(
        wdm.reshape(4, 128, M).transpose(1, 0, 2)).astype(BF16)
    common["WmT"] = np.ascontiguousarray(
        wm.T.astype(f32).reshape(8, 128, M).transpose(1, 0, 2)).astype(BF16)
    bias2 = wm @ np.asarray(inputs["bd"], np.float64) \
        + np.asarray(inputs["bm"], np.float64)
    common["bm"] = bias2.astype(f32).reshape(1, M).astype(BF16)

    sharded = {"xT": featmaj(inputs["x"], BF16)}
    for li in ("1", "2", "3"):
        sharded[f"h{li}T"] = featmaj(inputs[f"h{li}"], BF16)
        sharded[f"c{li}T"] = featmaj(inputs[f"c{li}"], f32)

    in_maps = []
    for c in range(ncore):
        m = dict(common)
        for k, v in sharded.items():
            m[k] = np.ascontiguousarray(v[:, :, c * bc:(c + 1) * bc])
        in_maps.append(m)
    return in_maps


def assemble_outputs(results, bc=B // N_CORES):
    ncore = len(results)
    f32 = np.float32

    def gather(name):
        out = np.empty((ncore * bc, L), f32)
        for c in range(ncore):
            out[c * bc:(c + 1) * bc] = results[c][name].reshape(L, bc).T
        return out

    mp = np.concatenate([results[c]["mp"].reshape(bc, M) for c in range(ncore)],
                        axis=0)
    return (mp, gather("y1T"), gather("nc1T"), gather("y2T"), gather("nc2T"),
            gather("y3T"), gather("nc3T"))


def run_on_cores(in_maps, bc=B // N_CORES):
    from concourse.bass_utils import run_bass_kernel_spmd
    nc = get_nc(bc)
    core_ids = list(range(len(in_maps)))
    try:
        res = run_bass_kernel_spmd(nc, in_maps, core_ids)
    except Exception:
        # transient device wedge ("mesh desynced") recovers on retry
        res = run_bass_kernel_spmd(nc, in_maps, core_ids)
    return res.results


def kernel(**inputs):
    bc = B // N_CORES
    in_maps = prep_inputs(inputs, bc)
    results = run_on_cores(in_maps, bc)
    return assemble_outputs(results, bc)


# revision 28
# speedup vs baseline: 1.2049x; 1.0845x over previous
"""Trainium2 Bass kernel: 3-layer LSTMCell stack + skip dense + log_softmax.

Data-parallel across 8 NeuronCores: batch (8192) sharded into 1024 rows per
core, weights replicated. All matmuls run in a feature-major ("transposed")
layout so the contraction dim sits on SBUF partitions with no on-chip
transposes: activations live as (feature, batch) tiles, weights are
pre-arranged on the host into lhsT slabs. bf16 matmul inputs, fp32
accumulation and elementwise math.
"""

import sys

for _p in ("/opt/trn_rl_repo",):
    if _p not in sys.path:
        sys.path.insert(0, _p)

import ml_dtypes
import numpy as np

N_CORES = 8
B, S, L, M = 8192, 512, 1024, 512
BF16 = np.float16  # fp16: same PE rate as bf16, 8x the mantissa precision

_CACHE = {}


def _build(bc, reps=1, diag=None):
    """Build + compile the per-core SPMD Bass program (batch slice = bc).

    reps > 1 wraps the whole body in an on-device loop (for timing: device
    wall-time scales with reps while the fixed dispatch overhead does not).
    diag="mm_only" emits just the DMAs+matmuls (wrong outputs; PE-span probe).
    """
    import contextlib

    import concourse.tile as tile
    import concourse.mybir as mybir
    from concourse import bacc

    mm_only = diag == "mm_only"

    f32 = mybir.dt.float32
    bf16 = mybir.dt.float16
    AF = mybir.ActivationFunctionType

    nch = max(1, bc // 512)  # batch chunks (PSUM free dim <= 512 fp32)
    ch = bc // nch
    nbt = bc // 128          # batch tiles for the final batch-major stage

    nc = bacc.Bacc(None, target_bir_lowering=False, debug=False,
                   num_devices=N_CORES)

    KT = [12, 16, 16]  # contraction k-tiles per layer ((S+L)/128, (L+L)/128 x2)

    xT = nc.dram_tensor("xT", [4, 128, bc], bf16, kind="ExternalInput")
    hT, cT, W, bias, yT, ncT = [], [], [], [], [], []
    for li in range(3):
        hT.append(nc.dram_tensor(f"h{li + 1}T", [8, 128, bc], bf16, kind="ExternalInput"))
        cT.append(nc.dram_tensor(f"c{li + 1}T", [8, 128, bc], f32, kind="ExternalInput"))
        W.append(nc.dram_tensor(f"W{li + 1}", [8, 128, 4, KT[li], 128], bf16, kind="ExternalInput"))
        bias.append(nc.dram_tensor(f"b{li + 1}", [128, 32], f32, kind="ExternalInput"))
        yT.append(nc.dram_tensor(f"y{li + 1}T", [8, 128, bc], f32, kind="ExternalOutput"))
        ncT.append(nc.dram_tensor(f"nc{li + 1}T", [8, 128, bc], f32, kind="ExternalOutput"))
    # host-folded dense stages: motors = x @ (Wm@Wd).T + y3 @ Wm.T + bias2
    WdmT = nc.dram_tensor("WdmT", [128, 4, M], bf16, kind="ExternalInput")
    WmT = nc.dram_tensor("WmT", [128, 8, M], bf16, kind="ExternalInput")
    bm = nc.dram_tensor("bm", [1, M], bf16, kind="ExternalInput")
    MP = nc.dram_tensor("mp", [nbt, 128, M], f32, kind="ExternalOutput")

    with tile.TileContext(nc) as tc:
        with tc.tile_pool(name="sb", bufs=1) as sb, \
             tc.tile_pool(name="ps", bufs=2, space="PSUM") as pspool, \
             (tc.For_i(0, reps, 1) if reps > 1 else contextlib.nullcontext()):

            # ---- persistent small/constant tiles -------------------------
            xt = []
            for kt in range(4):
                t = sb.tile([128, bc], bf16, tag=f"x{kt}", name=f"xt{kt}")
                nc.sync.dma_start(out=t, in_=xT[kt])
                xt.append(t)
            b_t = []
            for li in range(3):
                t = sb.tile([128, 32], f32, tag=f"bias{li}", name=f"bt{li}")
                nc.sync.dma_start(out=t, in_=bias[li][:, :])
                b_t.append(t)
            wdm_t = sb.tile([128, 4, M], bf16, tag="wdm", name="wdm_t")
            nc.sync.dma_start(out=wdm_t, in_=WdmT[:, :, :])
            wm_t = sb.tile([128, 8, M], bf16, tag="wm", name="wm_t")
            nc.sync.dma_start(out=wm_t, in_=WmT[:, :, :])
            bm_t = sb.tile([1, M], bf16, tag="bm", name="bm_t")
            nc.sync.dma_start(out=bm_t, in_=bm[:, :])
            ones_t = sb.tile([1, 128], bf16, tag="ones", name="ones_t")
            nc.vector.memset(ones_t, 1.0)

            # ---- the three LSTM cells ------------------------------------
            ybf_prev = None
            mids = []
            for li in range(3):
                ht = []
                for kt in range(8):
                    t = sb.tile([128, bc], bf16, tag=f"h{kt}", bufs=2,
                                name=f"h{li}_{kt}")
                    nc.sync.dma_start(out=t, in_=hT[li][kt])
                    ht.append(t)
                # h-tiles first: the previous layer's y tiles become ready
                # lb-by-lb, so consume them late in each accumulation group
                rhs = ht + (xt if li == 0 else ybf_prev)
                K = KT[li]
                ybf_new = []
                for lb in range(8):
                    w = sb.tile([128, 4, K, 128], bf16, tag="wslab", bufs=2,
                                name=f"w{li}_{lb}")
                    for g in range(4):
                        nc.sync.dma_start(out=w[:, g], in_=W[li][lb, :, g])
                    if not mm_only:
                        c_t = sb.tile([128, bc], f32, tag="c", bufs=3,
                                      name=f"c{li}_{lb}")
                        nc.sync.dma_start(out=c_t, in_=cT[li][lb])
                    cn = sb.tile([128, bc], f32, tag="cnew", bufs=2,
                                 name=f"cn{li}_{lb}")
                    hn = sb.tile([128, bc], f32, tag="hnew", bufs=2,
                                 name=f"hn{li}_{lb}")
                    yb = sb.tile([128, bc], bf16, tag=f"ybf{lb}", bufs=2,
                                 name=f"yb{li}_{lb}")
                    if mm_only:
                        nc.vector.memset(yb, 0.01)
                    for n in range(nch):
                        sl = slice(n * ch, (n + 1) * ch)
                        ps = [pspool.tile([128, ch], f32, tag=f"ps{g}",
                                          bufs=2, name=f"ps{li}_{lb}_{n}_{g}")
                              for g in range(4)]
                        for g in range(4):
                            for kt in range(K):
                                nc.tensor.matmul(ps[g], w[:, g, kt, :],
                                                 rhs[kt][:, sl],
                                                 start=(kt == 0),
                                                 stop=(kt == K - 1))
                        if mm_only:
                            continue
                        bb = b_t[li]
                        i_s = sb.tile([128, ch], f32, tag="i_s", bufs=2, name=f"i{li}_{lb}_{n}")
                        nc.scalar.activation(out=i_s, in_=ps[0], func=AF.Sigmoid,
                                             bias=bb[:, lb * 4 + 0:lb * 4 + 1])
                        f_s = sb.tile([128, ch], f32, tag="f_s", bufs=2, name=f"f{li}_{lb}_{n}")
                        nc.scalar.activation(out=f_s, in_=ps[1], func=AF.Sigmoid,
                                             bias=bb[:, lb * 4 + 1:lb * 4 + 2])
                        g_t = sb.tile([128, ch], f32, tag="g_t", bufs=2, name=f"g{li}_{lb}_{n}")
                        nc.scalar.activation(out=g_t, in_=ps[2], func=AF.Tanh,
                                             bias=bb[:, lb * 4 + 2:lb * 4 + 3])
                        o_s = sb.tile([128, ch], f32, tag="o_s", bufs=2, name=f"o{li}_{lb}_{n}")
                        nc.scalar.activation(out=o_s, in_=ps[3], func=AF.Sigmoid,
                                             bias=bb[:, lb * 4 + 3:lb * 4 + 4])
                        fc = sb.tile([128, ch], f32, tag="fc", bufs=2, name=f"fc{li}_{lb}_{n}")
                        nc.vector.tensor_mul(fc, f_s, c_t[:, sl])
                        ig = sb.tile([128, ch], f32, tag="ig", bufs=2, name=f"ig{li}_{lb}_{n}")
                        nc.vector.tensor_mul(ig, i_s, g_t)
                        nc.vector.tensor_add(cn[:, sl], fc, ig)
                        tct = sb.tile([128, ch], f32, tag="tct", bufs=2, name=f"tc{li}_{lb}_{n}")
                        nc.scalar.activation(out=tct, in_=cn[:, sl], func=AF.Tanh)
                        nc.vector.tensor_mul(hn[:, sl], o_s, tct)
                        nc.vector.tensor_copy(out=yb[:, sl], in_=hn[:, sl])
                    if not mm_only:
                        nc.sync.dma_start(out=ncT[li][lb], in_=cn)
                        nc.sync.dma_start(out=yT[li][lb], in_=hn)
                    ybf_new.append(yb)
                ybf_prev = ybf_new
            y3bf = ybf_prev

            # ---- motors (batch-major) + log_softmax ----------------------
            # motors = x @ (Wm@Wd).T + y3 @ Wm.T + (Wm@bd + bm), folded on
            # the host, so the x-side accumulation is ready from the start
            # and y3 tiles are consumed in the order layer 3 produces them.
            for bt in range(nbt):
                bsl = slice(bt * 128, (bt + 1) * 128)
                psm = pspool.tile([128, M], f32, tag="ps1", bufs=2, name=f"psm{bt}")
                for kt in range(4):
                    nc.tensor.matmul(psm, xt[kt][:, bsl], wdm_t[:, kt, :],
                                     start=(kt == 0), stop=False)
                for kt in range(8):
                    nc.tensor.matmul(psm, y3bf[kt][:, bsl], wm_t[:, kt, :],
                                     start=False, stop=False)
                nc.tensor.matmul(psm, ones_t, bm_t, start=False, stop=True)
                if mm_only:
                    continue
                mx = sb.tile([128, 1], f32, tag="mx", bufs=2, name=f"mx{bt}")
                nc.vector.tensor_reduce(mx, psm, axis=mybir.AxisListType.X,
                                        op=mybir.AluOpType.max, negate=True)
                e_t = sb.tile([128, M], f32, tag="e_t", bufs=2, name=f"e{bt}")
                se = sb.tile([128, 1], f32, tag="se", bufs=2, name=f"se{bt}")
                nc.scalar.activation(out=e_t, in_=psm, func=AF.Exp, bias=mx,
                                     accum_out=se)
                ls = sb.tile([128, 1], f32, tag="ls", bufs=2, name=f"ls{bt}")
                nc.scalar.activation(out=ls, in_=se, func=AF.Ln)
                sh = sb.tile([128, 1], f32, tag="sh", bufs=2, name=f"sh{bt}")
                nc.vector.tensor_sub(sh, mx, ls)
                mp_t = sb.tile([128, M], f32, tag="mp_t", bufs=2,
                               name=f"mp{bt}")
                nc.vector.tensor_scalar_add(mp_t, psm, sh)
                nc.sync.dma_start(out=MP[bt], in_=mp_t)

    nc.compile()
    return nc


def get_nc(bc=B // N_CORES, reps=1, diag=None):
    key = ("nc", bc, reps, diag)
    if key not in _CACHE:
        _CACHE[key] = _build(bc, reps, diag)
    return _CACHE[key]


def _arrange_gate_weights(Wih, Whh, kk):
    """-> (8, 128, 4, kk, 128) bf16 lhsT slabs: [lb, p, gate, kt, j] =
    Wcat[gate*1024 + lb*128 + j, kt*128 + p].

    Whh first: the kernel consumes the h-side k-tiles before the x/y-side
    ones (see `rhs = ht + ...` in _build)."""
    Wcat = np.concatenate([Whh, Wih], axis=1)  # (4096, K)
    a = Wcat.reshape(4, 8, 128, kk, 128)       # [gi, lb, j, kt, p]
    return np.ascontiguousarray(a.transpose(1, 4, 0, 3, 2)).astype(BF16)


def prep_inputs(inputs, bc=B // N_CORES):
    """Host-side prep: transpose/rearrange/cast. Returns per-core in_maps."""
    f32 = np.float32
    ncore = B // bc

    def featmaj(a, dt):  # (B, F) -> (F/128, 128, B) in dtype dt
        a = np.ascontiguousarray(np.asarray(a, f32).T)
        if dt is not f32:
            a = a.astype(dt)
        return a.reshape(a.shape[0] // 128, 128, B)

    common = {}
    common["W1"] = _arrange_gate_weights(inputs["Wih1"], inputs["Whh1"], 12)
    common["W2"] = _arrange_gate_weights(inputs["Wih2"], inputs["Whh2"], 16)
    common["W3"] = _arrange_gate_weights(inputs["Wih3"], inputs["Whh3"], 16)
    for li in ("1", "2", "3"):
        bsum = (np.asarray(inputs[f"bih{li}"], f32)
                + np.asarray(inputs[f"bhh{li}"], f32))
        common[f"b{li}"] = np.ascontiguousarray(
            bsum.reshape(4, 8, 128).transpose(2, 1, 0).reshape(128, 32))
    wd = np.asarray(inputs["Wd"], np.float64)   # (1024, 512)
    wm = np.asarray(inputs["Wm"], np.float64)   # (512, 1024)
    wdm = (wm @ wd).T.astype(f32)               # (512, 512) = (Wm@Wd).T
    common["WdmT"] = np.ascontiguousarray(
        wdm.reshape(4, 128, M).transpose(1, 0, 2)).astype(BF16)
    common["WmT"] = np.ascontiguousarray(
        wm.T.astype(f32).reshape(8, 128, M).transpose(1, 0, 2)).astype(BF16)
    bias2 = wm @ np.asarray(inputs["bd"], np.float64) \
        + np.asarray(inputs["bm"], np.float64)
    common["bm"] = bias2.astype(f32).reshape(1, M).astype(BF16)

    sharded = {"xT": featmaj(inputs["x"], BF16)}
    for li in ("1", "2", "3"):
        sharded[f"h{li}T"] = featmaj(inputs[f"h{li}"], BF16)
        sharded[f"c{li}T"] = featmaj(inputs[f"c{li}"], f32)

    in_maps = []
    for c in range(ncore):
        m = dict(common)
        for k, v in sharded.items():
            m[k] = np.ascontiguousarray(v[:, :, c * bc:(c + 1) * bc])
        in_maps.append(m)
    return in_maps


def assemble_outputs(results, bc=B // N_CORES):
    ncore = len(results)
    f32 = np.float32

    def gather(name):
        out = np.empty((ncore * bc, L), f32)
        for c in range(ncore):
            out[c * bc:(c + 1) * bc] = results[c][name].reshape(L, bc).T
        return out

    mp = np.concatenate([results[c]["mp"].reshape(bc, M) for c in range(ncore)],
                        axis=0)
    return (mp, gather("y1T"), gather("nc1T"), gather("y2T"), gather("nc2T"),
            gather("y3T"), gather("nc3T"))


def run_on_cores(in_maps, bc=B // N_CORES):
    from concourse.bass_utils import run_bass_kernel_spmd
    nc = get_nc(bc)
    core_ids = list(range(len(in_maps)))
    try:
        res = run_bass_kernel_spmd(nc, in_maps, core_ids)
    except Exception:
        # transient device wedge ("mesh desynced") recovers on retry
        res = run_bass_kernel_spmd(nc, in_maps, core_ids)
    return res.results


def kernel(**inputs):
    bc = B // N_CORES
    in_maps = prep_inputs(inputs, bc)
    results = run_on_cores(in_maps, bc)
    return assemble_outputs(results, bc)
